# revision 1
# baseline (speedup 1.0000x reference)
"""Bottom-up HTMM forward (nn_BottomUpHTMM) on 8 Trainium2 NeuronCores.

Sharding: data-parallel over the 128 trees of the forest — each of the 8
cores processes 16 trees (nodes contiguous per tree); the small
lambda_A/B/Pi/SP parameter tensors are replicated.  Forward only, so there
are no collectives; per-core (16, 8) outputs are concatenated on the host.

Device layout: partition dim = g*16 + c (G=8 generators x C=16 states);
free dim = nodes, level-major.  Children of in-level parent position P sit
at child-level positions 2P / 2P+1, so the per-level recurrence

    t    = softmax(A0)^T @ beta_left + softmax(A1)^T @ beta_right   (PE)
    b_u  = t * B[:, x]            (B columns fetched by GPSIMD ap_gather)
    nu   = mask^T @ b_u           (sum over c within each g block, PE)
    ll   = Ln(nu); recip = Exp(-ll)                                 (ACT)
    beta = b_u * (expand^T @ recip)                                 (PE+DVE)

runs on full-width tiles.  Per-tree log-likelihood = per-tree sum of ll
via ACT accum_out.
"""

from contextlib import ExitStack

import numpy as np

import concourse.mybir as mybir
import concourse.tile as tile
from concourse import bacc
from concourse._compat import with_exitstack
from concourse.bass_utils import run_bass_kernel_spmd

F32 = mybir.dt.float32
AF = mybir.ActivationFunctionType
ALU = mybir.AluOpType

C, M, L, G = 16, 256, 2, 8
DEPTH = 10
T = 16                      # trees per core
N_CORES = 8
NODES_PER_TREE = 2**DEPTH - 1          # 1023
CHUNK = 512
LVL_SIZE = [T * 2**k for k in range(DEPTH)]          # nodes per level per core
LVL_BASE = [T * (2**k - 1) for k in range(DEPTH)]    # level-major base offset
N_C = T * NODES_PER_TREE                              # 16368
GATHER_CHUNK = 2048


# --------------------------------------------------------------------------
# host-side input prep (pure layout: transpose / replicate / constant fill)
# --------------------------------------------------------------------------

def prep_shared_inputs(lambda_A, lambda_B, lambda_Pi, lambda_SP):
    lambda_A = np.asarray(lambda_A, dtype=np.float32)
    lambda_B = np.asarray(lambda_B, dtype=np.float32)
    lambda_Pi = np.asarray(lambda_Pi, dtype=np.float32)
    lambda_SP = np.asarray(lambda_SP, dtype=np.float32)

    # A_raw_l[g*16+b, g*16+a] = lambda_A[a, b, l, g]; off-block -87 (exp->~0)
    lamA = np.full((2, 128, 128), -87.0, dtype=np.float32)
    for g in range(G):
        for l in range(L):
            lamA[l, g * 16:(g + 1) * 16, g * 16:(g + 1) * 16] = \
                lambda_A[:, :, l, g].T
    lamB = np.ascontiguousarray(lambda_B.transpose(2, 0, 1).reshape(128, M))
    lamPi = np.ascontiguousarray(lambda_Pi.transpose(2, 0, 1).reshape(128, L))
    lamSP = np.ascontiguousarray(
        np.repeat(lambda_SP.T[:, None, :], C, axis=1).reshape(128, L))
    mask = np.zeros((128, G), dtype=np.float32)
    for g in range(G):
        mask[g * 16:(g + 1) * 16, g] = 1.0
    expand = np.ascontiguousarray(mask.T)
    return {
        "lamA0": np.ascontiguousarray(lamA[0]),
        "lamA1": np.ascontiguousarray(lamA[1]),
        "lamB": lamB,
        "lamPi": lamPi,
        "lamSP": lamSP,
        "mask": mask,
        "expand": expand,
    }


def prep_core_idx(x_core):
    """One core's (T, 1023) x slice -> (128, 1023) int16 wrapped indices."""
    cols = []
    for k in range(DEPTH):
        n_k = 2**k
        vec = np.ascontiguousarray(
            x_core[:, n_k - 1:2 * n_k - 1]).reshape(-1)
        w = vec.reshape(-1, 16).T
        cols.append(np.tile(w, (8, 1)))
    idx = np.concatenate(cols, axis=1).astype(np.int16)
    assert idx.shape == (128, N_C // 16)
    return idx


# --------------------------------------------------------------------------
# the tile kernel (per core; SPMD identical program, different inputs)
# --------------------------------------------------------------------------

@with_exitstack
def htmm_kernel(ctx: ExitStack, tc: tile.TileContext, outs, ins):
    nc = tc.nc

    const = ctx.enter_context(tc.tile_pool(name="const", bufs=1))
    small = ctx.enter_context(tc.tile_pool(name="small", bufs=2))
    betap = ctx.enter_context(tc.tile_pool(name="beta", bufs=1))
    bup = ctx.enter_context(tc.tile_pool(name="bu", bufs=4))
    recp = ctx.enter_context(tc.tile_pool(name="recip", bufs=4))
    gath = ctx.enter_context(tc.tile_pool(name="gath", bufs=4))
    llsc = ctx.enter_context(tc.tile_pool(name="llscratch", bufs=2))
    ps_t = ctx.enter_context(tc.tile_pool(name="ps_t", bufs=2, space="PSUM"))
    ps_nu = ctx.enter_context(tc.tile_pool(name="ps_nu", bufs=2, space="PSUM"))
    ps_rx = ctx.enter_context(tc.tile_pool(name="ps_rx", bufs=2, space="PSUM"))

    def load(name, shape, dtype=F32):
        t = const.tile(shape, dtype, tag=name, name=name)
        nc.gpsimd.dma_start(out=t[:], in_=ins[name])
        return t

    lamA = [load("lamA0", [128, 128]), load("lamA1", [128, 128])]
    lamB = load("lamB", [128, M])
    lamPi = load("lamPi", [128, L])
    lamSP = load("lamSP", [128, L])
    mask = load("mask", [128, G])
    expand = load("expand", [G, 128])
    idx = load("idx", [128, N_C // 16], mybir.dt.int16)

    # ---- softmaxes -----------------------------------------------------
    SPe = small.tile([128, L], F32, tag="spe")
    nc.scalar.activation(out=SPe[:], in_=lamSP[:], func=AF.Exp)
    SPden = small.tile([128, 1], F32, tag="spden")
    nc.vector.tensor_reduce(SPden[:], SPe[:], axis=mybir.AxisListType.X,
                            op=ALU.add)
    SPrec = small.tile([128, 1], F32, tag="sprec")
    nc.vector.reciprocal(SPrec[:], SPden[:])
    SPn = const.tile([128, L], F32, tag="spn")
    nc.vector.tensor_scalar_mul(SPn[:], SPe[:], SPrec[:])

    As = []
    for l in range(2):
        Ae = const.tile([128, 128], F32, tag=f"Ae{l}", name=f"Ae{l}")
        nc.scalar.activation(out=Ae[:], in_=lamA[l][:], func=AF.Exp)
        den = small.tile([128, 1], F32, tag=f"aden{l}", name=f"aden{l}")
        nc.vector.tensor_reduce(den[:], Ae[:], axis=mybir.AxisListType.X,
                                op=ALU.add)
        rec = small.tile([128, 1], F32, tag=f"arec{l}", name=f"arec{l}")
        nc.vector.reciprocal(rec[:], den[:])
        Asl = const.tile([128, 128], F32, tag=f"As{l}", name=f"As{l}")
        nc.vector.tensor_scalar(
            out=Asl[:], in0=Ae[:], scalar1=rec[:], scalar2=SPn[:, l:l + 1],
            op0=ALU.mult, op1=ALU.mult)
        As.append(Asl)

    Be = const.tile([128, M], F32, tag="be")
    nc.scalar.activation(out=Be[:], in_=lamB[:], func=AF.Exp)
    Bden = small.tile([128, 1], F32, tag="bden")
    nc.vector.tensor_reduce(Bden[:], Be[:], axis=mybir.AxisListType.X,
                            op=ALU.add)
    Brec = small.tile([128, 1], F32, tag="brec")
    nc.vector.reciprocal(Brec[:], Bden[:])
    B_r = const.tile([128, M], F32, tag="br")
    nc.vector.tensor_scalar_mul(B_r[:], Be[:], Brec[:])

    Pie = const.tile([128, L], F32, tag="pie")
    nc.scalar.activation(out=Pie[:], in_=lamPi[:], func=AF.Exp)
    nu_pi = ps_nu.tile([G, L], F32, tag="nu")
    nc.tensor.matmul(out=nu_pi[:], lhsT=mask[:], rhs=Pie[:])
    rec_pi = small.tile([G, L], F32, tag="recpi")
    nc.vector.reciprocal(rec_pi[:], nu_pi[:])
    rexp_pi = ps_rx.tile([128, L], F32, tag="rexp")
    nc.tensor.matmul(out=rexp_pi[:], lhsT=expand[:], rhs=rec_pi[:])
    Pi_col = const.tile([128, L], F32, tag="picol")
    nc.vector.tensor_mul(Pi_col[:], Pie[:], rexp_pi[:])

    # ---- ll buffer (tree-major) + beta tiles ---------------------------
    ll_buf = const.tile([G, T * NODES_PER_TREE], F32, tag="llbuf")

    beta_tiles = {}
    for k in range(1, DEPTH):
        n_ch = max(1, LVL_SIZE[k] // CHUNK)
        sz = min(CHUNK, LVL_SIZE[k])
        for i in range(n_ch):
            beta_tiles[(k, i)] = betap.tile(
                [128, sz], F32, tag=f"beta_{k}_{i}", name=f"beta_{k}_{i}")

    # ---- per-level processing (bottom-up) ------------------------------
    for k in range(DEPTH - 1, -1, -1):
        lsz = LVL_SIZE[k]
        n_ch = max(1, lsz // CHUNK)
        csz = min(CHUNK, lsz)
        n_k = 2**k
        trees_per_chunk = max(1, csz // n_k)

        n_g = max(1, lsz // GATHER_CHUNK)
        gsz = min(GATHER_CHUNK, lsz)
        bg_tiles = []
        for gi in range(n_g):
            bg = gath.tile([128, gsz], F32, tag="bg", name=f"bg_{k}_{gi}")
            off16 = (LVL_BASE[k] + gi * GATHER_CHUNK) // 16
            nc.gpsimd.ap_gather(
                out_ap=bg[:], in_ap=B_r[:],
                idxs_ap=idx[:, off16:off16 + gsz // 16],
                channels=128, num_elems=M, d=1, num_idxs=gsz)
            bg_tiles.append(bg)

        for i in range(n_ch):
            bgt = bg_tiles[i * CHUNK // GATHER_CHUNK]
            goff = (i * CHUNK) % GATHER_CHUNK
            bg_c = bgt[:, goff:goff + csz]

            b_u = bup.tile([128, csz], F32, tag="bu", name=f"bu_{k}_{i}")

            if k == DEPTH - 1:
                bg_r = bg_c.rearrange("p (j s) -> p s j", s=2)
                bu_r = b_u[:].rearrange("p (j s) -> p s j", s=2)
                for s in range(2):
                    nc.scalar.activation(
                        out=bu_r[:, s, :], in_=bg_r[:, s, :], func=AF.Copy,
                        scale=Pi_col[:, s:s + 1])
            else:
                t_ps = ps_t.tile([128, csz], F32, tag="tps",
                                 name=f"tps_{k}_{i}")
                kc = k + 1
                if LVL_SIZE[kc] <= CHUNK:
                    ct = beta_tiles[(kc, 0)]
                    r = ct[:, :2 * csz].rearrange("p (j s) -> p s j", s=2)
                    nc.tensor.matmul(out=t_ps[:], lhsT=As[0][:],
                                     rhs=r[:, 0, :], start=True, stop=False)
                    nc.tensor.matmul(out=t_ps[:], lhsT=As[1][:],
                                     rhs=r[:, 1, :], start=False, stop=True)
                else:
                    h = csz // 2
                    for hh in range(2):
                        ct = beta_tiles[(kc, 2 * i + hh)]
                        r = ct[:].rearrange("p (j s) -> p s j", s=2)
                        o = t_ps[:, hh * h:(hh + 1) * h]
                        nc.tensor.matmul(out=o, lhsT=As[0][:],
                                         rhs=r[:, 0, :], start=True,
                                         stop=False)
                        nc.tensor.matmul(out=o, lhsT=As[1][:],
                                         rhs=r[:, 1, :], start=False,
                                         stop=True)
                nc.vector.tensor_mul(b_u[:], t_ps[:], bg_c)

            nu_ps = ps_nu.tile([G, csz], F32, tag="nu", name=f"nu_{k}_{i}")
            nc.tensor.matmul(out=nu_ps[:], lhsT=mask[:], rhs=b_u[:])

            t0 = i * trees_per_chunk
            ll_view = ll_buf[:].rearrange(
                "p (t j) -> p t j", t=T)[:, t0:t0 + trees_per_chunk,
                                         n_k - 1:2 * n_k - 1]
            nu_3d = nu_ps[:].rearrange("p (t j) -> p t j", t=trees_per_chunk)
            nc.scalar.activation(out=ll_view, in_=nu_3d, func=AF.Ln)

            if k == 0:
                continue

            rec_sb = recp.tile([G, csz], F32, tag="recip",
                               name=f"recip_{k}_{i}")
            nc.scalar.activation(
                out=rec_sb[:].rearrange("p (t j) -> p t j",
                                        t=trees_per_chunk),
                in_=ll_view, func=AF.Exp, scale=-1.0)

            rexp = ps_rx.tile([128, csz], F32, tag="rexp",
                              name=f"rexp_{k}_{i}")
            nc.tensor.matmul(out=rexp[:], lhsT=expand[:], rhs=rec_sb[:])
            nc.vector.tensor_mul(beta_tiles[(k, i)][:], b_u[:], rexp[:])

    # ---- per-tree ll sums ---------------------------------------------
    out_sb = const.tile([G, T], F32, tag="outsb")
    for t in range(T):
        scr = llsc.tile([G, NODES_PER_TREE], F32, tag="llscratch",
                        name=f"llscr_{t}")
        nc.scalar.activation(
            out=scr[:],
            in_=ll_buf[:, t * NODES_PER_TREE:(t + 1) * NODES_PER_TREE],
            func=AF.Copy, accum_out=out_sb[:, t:t + 1])
    nc.gpsimd.dma_start(out=outs["out"].rearrange("t g -> g t"),
                        in_=out_sb[:])


# --------------------------------------------------------------------------
# driver
# --------------------------------------------------------------------------

_IN_SPECS = [
    ("lamA0", (128, 128), F32),
    ("lamA1", (128, 128), F32),
    ("lamB", (128, M), F32),
    ("lamPi", (128, L), F32),
    ("lamSP", (128, L), F32),
    ("mask", (128, G), F32),
    ("expand", (G, 128), F32),
    ("idx", (128, N_C // 16), mybir.dt.int16),
]

_NC_CACHE = {}


def build_bass():
    if "nc" in _NC_CACHE:
        return _NC_CACHE["nc"]
    nc = bacc.Bacc("TRN2", target_bir_lowering=False, debug=False,
                   num_devices=N_CORES)
    ins = {name: nc.dram_tensor(name, list(shape), dt, kind="ExternalInput").ap()
           for name, shape, dt in _IN_SPECS}
    out = nc.dram_tensor("out", [T, G], F32, kind="ExternalOutput").ap()
    with tile.TileContext(nc) as tc:
        htmm_kernel(tc, {"out": out}, ins)
    nc.compile()
    _NC_CACHE["nc"] = nc
    return nc


def run(inputs, trace=False, **kw):
    lambda_A = np.asarray(inputs["lambda_A"], np.float32)
    lambda_B = np.asarray(inputs["lambda_B"], np.float32)
    lambda_Pi = np.asarray(inputs["lambda_Pi"], np.float32)
    lambda_SP = np.asarray(inputs["lambda_SP"], np.float32)
    x = np.asarray(inputs["x"], np.int32)

    shared = prep_shared_inputs(lambda_A, lambda_B, lambda_Pi, lambda_SP)
    x2 = x.reshape(N_CORES * T, NODES_PER_TREE)
    in_maps = [
        dict(shared, idx=prep_core_idx(x2[c * T:(c + 1) * T]))
        for c in range(N_CORES)
    ]

    nc = build_bass()
    res = run_bass_kernel_spmd(nc, in_maps, core_ids=list(range(N_CORES)),
                               trace=trace, **kw)
    out = np.concatenate([r["out"] for r in res.results], axis=0)
    return out, res


def kernel(**inputs) -> np.ndarray:
    out, _ = run(inputs)
    return out
